# revision 11
# baseline (speedup 1.0000x reference)
"""AdMSoftmaxLoss (unique-label branch) on 8 TRN2 NeuronCores.

reference:
    G12 = x1 @ x2.T            # [N, N]
    x12 = G12 / ||G12 rows||   # row-normalized similarity
    L12[i] = num_i - log(exp(num_i) + sum_{j != i} exp(S * x12[i, j]))
      with num_i = S * (x12[i, i] - M)
    (symmetric for x21 = row-normalize(x2 @ x1.T))
    loss = -mean(L12) - mean(L21)

Sharding: data-parallel over rows; core c owns rows [c*N/8, (c+1)*N/8) of both
directions.  Each core holds the full transposed views of x1/x2 so its rows are
complete; no collectives are needed, the final mean is done on host.

Key device-side structure (per core):
  - Row norms without materializing G: ||G12[i,:]||^2 = x1_i^T (X2^T X2) x1_i.
    The [256,256] Gram matrices C2 = X2^T X2 and C1 = X1^T X1 are computed by
    streaming the natural-layout x over the TensorEngine.
  - Diagonal G[i,i] = x1_i . x2_i via a rowwise dot (tensor_tensor_reduce).
  - Main matmul streams G row-tiles through PSUM in [128, 2048] groups; the
    ScalarEngine applies exp(scale_i * g) straight out of PSUM (per-partition
    scale = S/n_i) with accum_out producing the row sums; G is never stored.
"""

import math

import numpy as np

import concourse.bacc as bacc
from concourse import mybir
from concourse.bass import ts
from concourse.bass_utils import run_bass_kernel_spmd
from concourse.tile import TileContext

P = 128          # partitions
D = 256          # feature dim
KH = D // P      # k-halves of the contraction dim
S = 1.0          # AdMSoftmax scale
MARGIN = 0.4     # AdMSoftmax margin
F32 = mybir.dt.float32
N_CORES = 8

Alu = mybir.AluOpType
Act = mybir.ActivationFunctionType


def build(NF=8192, NL=1024, CG=2048, stage="full"):
    LVL = {"gram": 0, "ssq": 1, "scale": 2, "main": 3, "full": 4}[stage]
    """Build the SPMD graph for one core (all cores run the same graph).

    NF: total rows (columns of each local G row-block)
    NL: rows owned by this core
    CG: PSUM group width for the main matmul (multiple of 512, CG*2 <= 4096)
    """
    NT = NL // P          # row tiles per direction
    NG = NF // CG         # psum groups per row tile
    NC4 = CG // 512       # matmuls per group per k-half
    JT = 4                # natural-layout j-tiles per streamed DMA
    NJ = NF // (P * JT)

    nc = bacc.Bacc("TRN2", target_bir_lowering=False, debug=False,
                   num_devices=N_CORES)

    a_fT = nc.declare_dram_parameter("a_fT", [D, NF], F32, isOutput=False)
    b_fT = nc.declare_dram_parameter("b_fT", [D, NF], F32, isOutput=False)
    a_lT = nc.declare_dram_parameter("a_lT", [D, NL], F32, isOutput=False)
    b_lT = nc.declare_dram_parameter("b_lT", [D, NL], F32, isOutput=False)
    a_l = nc.declare_dram_parameter("a_l", [NL, D], F32, isOutput=False)
    b_l = nc.declare_dram_parameter("b_l", [NL, D], F32, isOutput=False)
    a_f = nc.declare_dram_parameter("a_f", [NF, D], F32, isOutput=False)
    b_f = nc.declare_dram_parameter("b_f", [NF, D], F32, isOutput=False)
    out = nc.declare_dram_parameter("out", [P, 2, NT], F32, isOutput=True)

    with TileContext(nc) as tc:
        with tc.tile_pool(name="res", bufs=1) as res, \
             tc.tile_pool(name="small", bufs=2) as small:

            # ---- resident inputs ----
            alT = res.tile([P, KH, NL], F32, tag="alT")
            blT = res.tile([P, KH, NL], F32, tag="blT")
            al = res.tile([P, NT, D], F32, tag="al")
            bl = res.tile([P, NT, D], F32, tag="bl")
            nc.sync.dma_start(out=al, in_=a_l[:, :].rearrange("(t p) d -> p t d", p=P))
            nc.sync.dma_start(out=bl, in_=b_l[:, :].rearrange("(t p) d -> p t d", p=P))
            for h in range(KH):
                nc.sync.dma_start(out=alT[:, h, :], in_=a_lT[ts(h, P), :])
                nc.sync.dma_start(out=blT[:, h, :], in_=b_lT[ts(h, P), :])

            afT = res.tile([P, KH, NF], F32, tag="afT")
            bfT = res.tile([P, KH, NF], F32, tag="bfT")
            for h in range(KH):
                nc.sync.dma_start(out=bfT[:, h, :], in_=b_fT[ts(h, P), :])
                nc.sync.dma_start(out=afT[:, h, :], in_=a_fT[ts(h, P), :])

            c2 = res.tile([P, KH, D], F32, tag="c2sb")
            c1 = res.tile([P, KH, D], F32, tag="c1sb")
            ssq = res.tile([P, 2, NT], F32, tag="ssq")
            dd = res.tile([P, NT], F32, tag="dd")

            # ---- Gram matrices C2 = X2^T X2, C1 = X1^T X1 (stream) ----
            with tc.tile_pool(name="cps", bufs=1, space="PSUM") as cps, \
                 tc.tile_pool(name="yps", bufs=2, space="PSUM") as yps, \
                 tc.tile_pool(name="stream", bufs=3) as stream:
                for nm, src, dst in (("c2", b_f, c2), ("c1", a_f, c1)):
                    c_ps = [cps.tile([P, D], F32, tag=f"{nm}{h}", name=f"cps_{nm}{h}")
                            for h in range(KH)]
                    srcr = src[:, :].rearrange("(v t p) d -> v p t d", p=P, t=JT)
                    for v in range(NJ):
                        st = stream.tile([P, JT, D], F32, tag="stream")
                        nc.sync.dma_start(out=st, in_=srcr[v])
                        for t in range(JT):
                            for h in range(KH):
                                nc.tensor.matmul(
                                    c_ps[h],
                                    lhsT=st[:, t, ts(h, P)],
                                    rhs=st[:, t, :],
                                    start=(v == 0 and t == 0),
                                    stop=(v == NJ - 1 and t == JT - 1),
                                )
                    for h in range(KH):
                        nc.vector.tensor_copy(dst[:, h, :], c_ps[h])

                # ---- Y = x_l @ C ; ssq_i = x_l_i . Y_i ; dd_i = x1_i . x2_i ----
                for di, (lt, nat, cc) in enumerate(
                        ((alT, al, c2), (blT, bl, c1)) if LVL >= 1 else ()):
                    for t in range(NT):
                        yp = yps.tile([P, D], F32, tag="yp")
                        for h in range(KH):
                            nc.tensor.matmul(
                                yp,
                                lhsT=lt[:, h, ts(t, P)],
                                rhs=cc[:, h, :],
                                start=(h == 0),
                                stop=(h == KH - 1),
                            )
                        o = small.tile([P, D], F32, tag="scr")
                        nc.vector.tensor_tensor(o, nat[:, t, :], yp, Alu.mult)
                        nc.vector.tensor_reduce(out=ssq[:, di, t:t + 1], in_=o,
                                                axis=mybir.AxisListType.X,
                                                op=Alu.add)
                for t in range(NT if LVL >= 1 else 0):
                    o = small.tile([P, D], F32, tag="scr")
                    nc.vector.tensor_tensor(o, al[:, t, :], bl[:, t, :], Alu.mult)
                    nc.vector.tensor_reduce(out=dd[:, t:t + 1], in_=o,
                                            axis=mybir.AxisListType.X,
                                            op=Alu.add)

            # ---- scale_i = S / sqrt(ssq_i), Newton-refined ----
            nrm = res.tile([P, 2, NT], F32, tag="nrm")
            rin = res.tile([P, 2, NT], F32, tag="rin")
            nt1 = res.tile([P, 2, NT], F32, tag="nt1")
            if LVL >= 2:
                nc.scalar.sqrt(nrm, ssq)
                nc.vector.reciprocal(rin, nrm)
                for _ in range(2):
                    nc.vector.tensor_mul(nt1, rin, rin)
                    nc.vector.tensor_mul(nt1, nt1, ssq)
                    nc.vector.tensor_scalar(out=nt1, in0=nt1, scalar1=-0.5,
                                            scalar2=1.5, op0=Alu.mult, op1=Alu.add)
                    nc.vector.tensor_mul(rin, rin, nt1)
                if S != 1.0:
                    nc.vector.tensor_scalar(out=rin, in0=rin, scalar1=float(S),
                                            scalar2=None, op0=Alu.mult)

            # ---- main matmul: exp row sums, G streamed through PSUM ----
            epart = res.tile([P, 2, NT, NG], F32, tag="epart")
            scx = res.tile([P, CG], F32, tag="scx")
            with tc.tile_pool(name="mm", bufs=2, space="PSUM") as psmm:
                for di in range(2 if LVL >= 3 else 0):
                    lt = alT if di == 0 else blT
                    rt = bfT if di == 0 else afT
                    for t in range(NT):
                        for g in range(NG):
                            ps = psmm.tile([P, CG], F32, tag="ps")
                            for h in range(KH):
                                for c4 in range(NC4):
                                    nc.tensor.matmul(
                                        ps[:, ts(c4, 512)],
                                        lhsT=lt[:, h, ts(t, P)],
                                        rhs=rt[:, h, ts(g * NC4 + c4, 512)],
                                        start=(h == 0),
                                        stop=(h == KH - 1),
                                    )
                            nc.scalar.activation(
                                out=scx, in_=ps, func=Act.Exp,
                                scale=rin[:, di, t:t + 1],
                                accum_out=epart[:, di, t, g:g + 1],
                            )

            # ---- per-row tail ----
            # sim_ii = dd * (S/n); num = sim_ii - S*M
            # denom = exp(num) + rowsum(exp) - exp(sim_ii)
            #       = rowsum(exp) - (1 - exp(-S*M)) * exp(sim_ii)
            # L = num - log(denom)
            esum = res.tile([P, 2, NT], F32, tag="esum")
            sim = res.tile([P, 2, NT], F32, tag="sim")
            tt = res.tile([P, 2, NT], F32, tag="tt")
            t2 = res.tile([P, 2, NT], F32, tag="t2")
            lg = res.tile([P, 2, NT], F32, tag="lg")
            lv = res.tile([P, 2, NT], F32, tag="lv")
            if LVL == 0:
                nc.sync.dma_start(out=out[:, :, :], in_=c2[:, :, 0:NT])
            elif LVL == 1:
                nc.sync.dma_start(out=out[:, :, :], in_=ssq)
            elif LVL == 2:
                nc.sync.dma_start(out=out[:, :, :], in_=rin)
            elif LVL == 3:
                nc.sync.dma_start(out=out[:, :, :], in_=epart[:, :, :, 0])
            else:
                nc.vector.tensor_reduce(out=esum, in_=epart,
                                        axis=mybir.AxisListType.X, op=Alu.add)
                nc.vector.tensor_tensor(sim, rin,
                                        dd[:, None, :].to_broadcast([P, 2, NT]),
                                        Alu.mult)
                nc.scalar.activation(tt, sim, func=Act.Exp)
                nc.vector.tensor_scalar(out=t2, in0=tt,
                                        scalar1=-(1.0 - math.exp(-S * MARGIN)),
                                        scalar2=None, op0=Alu.mult)
                nc.vector.tensor_add(t2, t2, esum)
                nc.scalar.activation(lg, t2, func=Act.Ln)
                nc.vector.tensor_sub(lv, sim, lg)
                nc.vector.tensor_scalar(out=lv, in0=lv, scalar1=-S * MARGIN,
                                        scalar2=None, op0=Alu.add)
                nc.sync.dma_start(out=out[:, :, :], in_=lv)

    nc.compile()
    return nc


_CACHE = {}


def _get_nc(NF, NL):
    key = (NF, NL)
    if key not in _CACHE:
        _CACHE[key] = build(NF=NF, NL=NL, CG=min(2048, NF))
    return _CACHE[key]


def shard_inputs(x1, x2):
    N = x1.shape[0]
    NL = N // N_CORES
    x1T = np.ascontiguousarray(x1.T)
    x2T = np.ascontiguousarray(x2.T)
    in_maps = []
    for c in range(N_CORES):
        sl = slice(c * NL, (c + 1) * NL)
        in_maps.append({
            "a_fT": x1T, "b_fT": x2T,
            "a_lT": np.ascontiguousarray(x1T[:, sl]),
            "b_lT": np.ascontiguousarray(x2T[:, sl]),
            "a_l": np.ascontiguousarray(x1[sl]),
            "b_l": np.ascontiguousarray(x2[sl]),
            "a_f": x1, "b_f": x2,
        })
    return in_maps


def run(x1, x2, trace=False):
    x1 = np.ascontiguousarray(np.asarray(x1, np.float32))
    x2 = np.ascontiguousarray(np.asarray(x2, np.float32))
    N = x1.shape[0]
    NL = N // N_CORES
    nc = _get_nc(N, NL)
    res = run_bass_kernel_spmd(nc, shard_inputs(x1, x2),
                               core_ids=list(range(N_CORES)), trace=trace)
    NT = NL // P
    L12 = np.empty((N_CORES, NL), np.float32)
    L21 = np.empty((N_CORES, NL), np.float32)
    for c in range(N_CORES):
        o = np.asarray(res.results[c]["out"]).reshape(P, 2, NT)
        L12[c] = o[:, 0, :].T.reshape(NL)
        L21[c] = o[:, 1, :].T.reshape(NL)
    L12 = L12.reshape(N)
    L21 = L21.reshape(N)
    loss = np.float32(-(L12.mean(dtype=np.float64) + L21.mean(dtype=np.float64)))
    return (loss, L12, L21), res


def kernel(x1, x2, sentence_id=None, **_):
    (loss, L12, L21), _res = run(x1, x2, trace=False)
    return loss, L12, L21


# revision 12
# speedup vs baseline: 2.2047x; 2.2047x over previous
"""AdMSoftmaxLoss (unique-label branch) on 8 TRN2 NeuronCores.

reference:
    G12 = x1 @ x2.T            # [N, N]
    x12 = G12 / ||G12 rows||   # row-normalized similarity
    L12[i] = num_i - log(exp(num_i) + sum_{j != i} exp(S * x12[i, j]))
      with num_i = S * (x12[i, i] - M)
    (symmetric for x21 = row-normalize(x2 @ x1.T))
    loss = -mean(L12) - mean(L21)

Sharding: data-parallel over rows; core c owns rows [c*N/8, (c+1)*N/8) of both
directions.  Each core holds the full transposed views of x1/x2 so its rows are
complete; no collectives are needed, the final mean is done on host.

Key device-side structure (per core):
  - Row norms without materializing G: ||G12[i,:]||^2 = x1_i^T (X2^T X2) x1_i.
    The [256,256] Gram matrices C2 = X2^T X2 and C1 = X1^T X1 are computed by
    streaming the natural-layout x over the TensorEngine.
  - Diagonal G[i,i] = x1_i . x2_i via a rowwise dot (tensor_tensor_reduce).
  - Main matmul streams G row-tiles through PSUM in [128, 2048] groups; the
    ScalarEngine applies exp(scale_i * g) straight out of PSUM (per-partition
    scale = S/n_i) with accum_out producing the row sums; G is never stored.
"""

import math

import numpy as np

import concourse.bacc as bacc
from concourse import mybir
from concourse.bass import ts
from concourse.bass_utils import run_bass_kernel_spmd
from concourse.tile import TileContext

P = 128          # partitions
D = 256          # feature dim
KH = D // P      # k-halves of the contraction dim
S = 1.0          # AdMSoftmax scale
MARGIN = 0.4     # AdMSoftmax margin
F32 = mybir.dt.float32
BF16 = mybir.dt.bfloat16
N_CORES = 8

Alu = mybir.AluOpType
Act = mybir.ActivationFunctionType


def build(NF=8192, NL=1024, CG=2048, stage="full"):
    LVL = {"gram": 0, "ssq": 1, "scale": 2, "main": 3, "full": 4}[stage]
    """Build the SPMD graph for one core (all cores run the same graph).

    NF: total rows (columns of each local G row-block)
    NL: rows owned by this core
    CG: PSUM group width for the main matmul (multiple of 512, CG*2 <= 4096)
    """
    NT = NL // P          # row tiles per direction
    NG = NF // CG         # psum groups per row tile
    NC4 = CG // 512       # matmuls per group per k-half
    JT = 4                # natural-layout j-tiles per streamed DMA
    NJ = NF // (P * JT)

    nc = bacc.Bacc("TRN2", target_bir_lowering=False, debug=False,
                   num_devices=N_CORES)

    a_fT = nc.declare_dram_parameter("a_fT", [D, NF], BF16, isOutput=False)
    b_fT = nc.declare_dram_parameter("b_fT", [D, NF], BF16, isOutput=False)
    a_lT = nc.declare_dram_parameter("a_lT", [D, NL], BF16, isOutput=False)
    b_lT = nc.declare_dram_parameter("b_lT", [D, NL], BF16, isOutput=False)
    a_l = nc.declare_dram_parameter("a_l", [NL, D], F32, isOutput=False)
    b_l = nc.declare_dram_parameter("b_l", [NL, D], F32, isOutput=False)
    a_f = nc.declare_dram_parameter("a_f", [NF, D], BF16, isOutput=False)
    b_f = nc.declare_dram_parameter("b_f", [NF, D], BF16, isOutput=False)
    out = nc.declare_dram_parameter("out", [P, 2, NT], F32, isOutput=True)

    with TileContext(nc) as tc:
        with tc.tile_pool(name="res", bufs=1) as res, \
             tc.tile_pool(name="small", bufs=2) as small:

            # ---- resident inputs ----
            alT = res.tile([P, KH, NL], BF16, tag="alT")
            blT = res.tile([P, KH, NL], BF16, tag="blT")
            al = res.tile([P, NT, D], F32, tag="al")
            bl = res.tile([P, NT, D], F32, tag="bl")
            nc.sync.dma_start(out=al, in_=a_l[:, :].rearrange("(t p) d -> p t d", p=P))
            nc.sync.dma_start(out=bl, in_=b_l[:, :].rearrange("(t p) d -> p t d", p=P))
            for h in range(KH):
                nc.sync.dma_start(out=alT[:, h, :], in_=a_lT[ts(h, P), :])
                nc.sync.dma_start(out=blT[:, h, :], in_=b_lT[ts(h, P), :])

            afT = res.tile([P, KH, NF], BF16, tag="afT")
            bfT = res.tile([P, KH, NF], BF16, tag="bfT")
            for h in range(KH):
                nc.sync.dma_start(out=bfT[:, h, :], in_=b_fT[ts(h, P), :])
                nc.sync.dma_start(out=afT[:, h, :], in_=a_fT[ts(h, P), :])

            c2 = res.tile([P, KH, D], BF16, tag="c2sb")
            c1 = res.tile([P, KH, D], BF16, tag="c1sb")
            ssq = res.tile([P, 2, NT], F32, tag="ssq")
            dd = res.tile([P, NT], F32, tag="dd")

            # ---- Gram matrices C2 = X2^T X2, C1 = X1^T X1 (stream) ----
            with tc.tile_pool(name="cps", bufs=1, space="PSUM") as cps, \
                 tc.tile_pool(name="yps", bufs=2, space="PSUM") as yps, \
                 tc.tile_pool(name="stream", bufs=3) as stream:
                for nm, src, dst in (("c2", b_f, c2), ("c1", a_f, c1)):
                    c_ps = [cps.tile([P, D], F32, tag=f"{nm}{h}", name=f"cps_{nm}{h}")
                            for h in range(KH)]
                    srcr = src[:, :].rearrange("(v t p) d -> v p t d", p=P, t=JT)
                    for v in range(NJ):
                        st = stream.tile([P, JT, D], BF16, tag="stream")
                        nc.sync.dma_start(out=st, in_=srcr[v])
                        for t in range(JT):
                            for h in range(KH):
                                nc.tensor.matmul(
                                    c_ps[h],
                                    lhsT=st[:, t, ts(h, P)],
                                    rhs=st[:, t, :],
                                    start=(v == 0 and t == 0),
                                    stop=(v == NJ - 1 and t == JT - 1),
                                )
                    for h in range(KH):
                        nc.vector.tensor_copy(dst[:, h, :], c_ps[h])

                # ---- Y = x_l @ C ; ssq_i = x_l_i . Y_i ; dd_i = x1_i . x2_i ----
                for di, (lt, nat, cc) in enumerate(
                        ((alT, al, c2), (blT, bl, c1)) if LVL >= 1 else ()):
                    for t in range(NT):
                        yp = yps.tile([P, D], F32, tag="yp")
                        for h in range(KH):
                            nc.tensor.matmul(
                                yp,
                                lhsT=lt[:, h, ts(t, P)],
                                rhs=cc[:, h, :],
                                start=(h == 0),
                                stop=(h == KH - 1),
                            )
                        o = small.tile([P, D], F32, tag="scr")
                        nc.vector.tensor_tensor(o, nat[:, t, :], yp, Alu.mult)
                        nc.vector.tensor_reduce(out=ssq[:, di, t:t + 1], in_=o,
                                                axis=mybir.AxisListType.X,
                                                op=Alu.add)
                for t in range(NT if LVL >= 1 else 0):
                    o = small.tile([P, D], F32, tag="scr")
                    nc.vector.tensor_tensor(o, al[:, t, :], bl[:, t, :], Alu.mult)
                    nc.vector.tensor_reduce(out=dd[:, t:t + 1], in_=o,
                                            axis=mybir.AxisListType.X,
                                            op=Alu.add)

            # ---- scale_i = S / sqrt(ssq_i), Newton-refined ----
            nrm = res.tile([P, 2, NT], F32, tag="nrm")
            rin = res.tile([P, 2, NT], F32, tag="rin")
            nt1 = res.tile([P, 2, NT], F32, tag="nt1")
            if LVL >= 2:
                nc.scalar.sqrt(nrm, ssq)
                nc.vector.reciprocal(rin, nrm)
                for _ in range(2):
                    nc.vector.tensor_mul(nt1, rin, rin)
                    nc.vector.tensor_mul(nt1, nt1, ssq)
                    nc.vector.tensor_scalar(out=nt1, in0=nt1, scalar1=-0.5,
                                            scalar2=1.5, op0=Alu.mult, op1=Alu.add)
                    nc.vector.tensor_mul(rin, rin, nt1)
                if S != 1.0:
                    nc.vector.tensor_scalar(out=rin, in0=rin, scalar1=float(S),
                                            scalar2=None, op0=Alu.mult)

            # ---- main matmul: exp row sums, G streamed through PSUM ----
            epart = res.tile([P, 2, NT, NG], F32, tag="epart")
            scx = res.tile([P, CG], F32, tag="scx")
            with tc.tile_pool(name="mm", bufs=2, space="PSUM") as psmm:
                for di in range(2 if LVL >= 3 else 0):
                    lt = alT if di == 0 else blT
                    rt = bfT if di == 0 else afT
                    for t in range(NT):
                        for g in range(NG):
                            ps = psmm.tile([P, CG], F32, tag="ps")
                            for h in range(KH):
                                for c4 in range(NC4):
                                    nc.tensor.matmul(
                                        ps[:, ts(c4, 512)],
                                        lhsT=lt[:, h, ts(t, P)],
                                        rhs=rt[:, h, ts(g * NC4 + c4, 512)],
                                        start=(h == 0),
                                        stop=(h == KH - 1),
                                    )
                            nc.scalar.activation(
                                out=scx, in_=ps, func=Act.Exp,
                                scale=rin[:, di, t:t + 1],
                                accum_out=epart[:, di, t, g:g + 1],
                            )

            # ---- per-row tail ----
            # sim_ii = dd * (S/n); num = sim_ii - S*M
            # denom = exp(num) + rowsum(exp) - exp(sim_ii)
            #       = rowsum(exp) - (1 - exp(-S*M)) * exp(sim_ii)
            # L = num - log(denom)
            esum = res.tile([P, 2, NT], F32, tag="esum")
            sim = res.tile([P, 2, NT], F32, tag="sim")
            tt = res.tile([P, 2, NT], F32, tag="tt")
            t2 = res.tile([P, 2, NT], F32, tag="t2")
            lg = res.tile([P, 2, NT], F32, tag="lg")
            lv = res.tile([P, 2, NT], F32, tag="lv")
            if LVL == 0:
                nc.sync.dma_start(out=out[:, :, :], in_=c2[:, :, 0:NT])
            elif LVL == 1:
                nc.sync.dma_start(out=out[:, :, :], in_=ssq)
            elif LVL == 2:
                nc.sync.dma_start(out=out[:, :, :], in_=rin)
            elif LVL == 3:
                nc.sync.dma_start(out=out[:, :, :], in_=epart[:, :, :, 0])
            else:
                nc.vector.tensor_reduce(out=esum, in_=epart,
                                        axis=mybir.AxisListType.X, op=Alu.add)
                nc.vector.tensor_tensor(sim, rin,
                                        dd[:, None, :].to_broadcast([P, 2, NT]),
                                        Alu.mult)
                nc.scalar.activation(tt, sim, func=Act.Exp)
                nc.vector.tensor_scalar(out=t2, in0=tt,
                                        scalar1=-(1.0 - math.exp(-S * MARGIN)),
                                        scalar2=None, op0=Alu.mult)
                nc.vector.tensor_add(t2, t2, esum)
                nc.scalar.activation(lg, t2, func=Act.Ln)
                nc.vector.tensor_sub(lv, sim, lg)
                nc.vector.tensor_scalar(out=lv, in0=lv, scalar1=-S * MARGIN,
                                        scalar2=None, op0=Alu.add)
                nc.sync.dma_start(out=out[:, :, :], in_=lv)

    nc.compile()
    return nc


_CACHE = {}


def _get_nc(NF, NL):
    key = (NF, NL)
    if key not in _CACHE:
        _CACHE[key] = build(NF=NF, NL=NL, CG=min(2048, NF))
    return _CACHE[key]


def shard_inputs(x1, x2):
    import ml_dtypes
    bf = ml_dtypes.bfloat16
    N = x1.shape[0]
    NL = N // N_CORES
    x1b = x1.astype(bf)
    x2b = x2.astype(bf)
    x1T = np.ascontiguousarray(x1b.T)
    x2T = np.ascontiguousarray(x2b.T)
    in_maps = []
    for c in range(N_CORES):
        sl = slice(c * NL, (c + 1) * NL)
        in_maps.append({
            "a_fT": x1T, "b_fT": x2T,
            "a_lT": np.ascontiguousarray(x1T[:, sl]),
            "b_lT": np.ascontiguousarray(x2T[:, sl]),
            "a_l": np.ascontiguousarray(x1[sl]),
            "b_l": np.ascontiguousarray(x2[sl]),
            "a_f": x1b, "b_f": x2b,
        })
    return in_maps


def run(x1, x2, trace=False):
    x1 = np.ascontiguousarray(np.asarray(x1, np.float32))
    x2 = np.ascontiguousarray(np.asarray(x2, np.float32))
    N = x1.shape[0]
    NL = N // N_CORES
    nc = _get_nc(N, NL)
    res = run_bass_kernel_spmd(nc, shard_inputs(x1, x2),
                               core_ids=list(range(N_CORES)), trace=trace)
    NT = NL // P
    L12 = np.empty((N_CORES, NL), np.float32)
    L21 = np.empty((N_CORES, NL), np.float32)
    for c in range(N_CORES):
        o = np.asarray(res.results[c]["out"]).reshape(P, 2, NT)
        L12[c] = o[:, 0, :].T.reshape(NL)
        L21[c] = o[:, 1, :].T.reshape(NL)
    L12 = L12.reshape(N)
    L21 = L21.reshape(N)
    loss = np.float32(-(L12.mean(dtype=np.float64) + L21.mean(dtype=np.float64)))
    return (loss, L12, L21), res


def kernel(x1, x2, sentence_id=None, **_):
    (loss, L12, L21), _res = run(x1, x2, trace=False)
    return loss, L12, L21
